# revision 45
# baseline (speedup 1.0000x reference)
"""CapsuleLayer dynamic-routing kernel for Trainium2 (Bass/Tile), SPMD over 8 cores.

Math (per batch sample, from the reference):
    u_hat[j, (i,k)] = sum_k' x[j, k'] * W[k', (i,k)]        j=1024, k'=256, (i,k)=16x32=512
    b_0 = 0
    for t in 0..3:
        c = softmax_i(b)                                    [16, 1024]
        s[i, k] = sum_j c[i, j] * u_hat[j, (i,k)]
        v = s / sqrt(sum_k s^2 + eps)                       [16, 32]
        if t < 3: b[i, j] = sum_k v[i, k] * u_hat[j, (i,k)]
    return v

KEY STRUCTURE: u_hat is never materialized. Both routing contractions factor
through W:
    s = (C x) W          (contract j against x [256 wide], then k'=256 with W)
    b = (W v) . x        (tiny w_v = W^T-blocks . v pass, then k'=256 with xT)
This removes the u_hat matmuls, all PE transposes of u_hat, and the PSUM
evacuation traffic that dominated the direct implementation.

Sharding: data-parallel over batch (128 -> 16 per core), W replicated.

Per-core schedule: 4 groups x 4 samples (col-strip packed, tile_position),
4 routing rounds emitted stage-major round-robin across groups so every
engine queue (PE / DVE / ScalarE / sync-DMA) sees work in readiness order.
All transposes (cx->cxT, masked->maskedT, b->bT) are DMA xbar transposes
(SBUF->SBUF) on the sync queue - zero PE time. xT is pre-transposed on the
host so both x layouts stream from DRAM with plain DMAs.
"""

import functools

import numpy as np

import concourse.bass as bass
import concourse.mybir as mybir
import concourse.tile as tile
from concourse import bacc
from concourse.bass_utils import run_bass_kernel_spmd

F32 = mybir.dt.float32
I32 = mybir.dt.int32
F16 = mybir.dt.float16
AF = mybir.ActivationFunctionType
ALU = mybir.AluOpType
AX = mybir.AxisListType
ts = bass.ts

NCORES = 8
BFULL = 128
BSH = BFULL // NCORES  # 16 samples per core
NJ, NK, ND = 1024, 256, 512  # j, k', (i,k)
NI, DK = 16, 32
JT, KT, IKT = NJ // 128, NK // 128, ND // 128  # 8, 2, 4
GS = 4  # samples per routing group (packed in PSUM partitions at 32-stride)
NG = BSH // GS  # 4
ROUTINGS = 4
EPS = 1e-7
P = 128


def _build_body(nc, tc, x_ap, w_ap, mask_ap, sel_ap, ident_ap, out_ap, ctx):
    consts = ctx.enter_context(tc.tile_pool(name="consts", bufs=1))
    xp = ctx.enter_context(tc.tile_pool(name="xp", bufs=NG))
    sm = ctx.enter_context(tc.tile_pool(name="sm", bufs=2 * NG))
    rt = ctx.enter_context(tc.tile_pool(name="rt", bufs=4))
    psum = ctx.enter_context(tc.tile_pool(name="psum", bufs=2, space="PSUM"))

    # ---- constants (w32 first: the wf cast gates wT + C-stage) ----
    w32 = consts.tile([P, KT, ND], F32)
    nc.sync.dma_start(w32[:], w_ap.rearrange("(t p) d -> p t d", p=P))
    maskT = consts.tile([P, IKT, P], F16)
    nc.sync.dma_start(maskT[:], mask_ap)
    sel = consts.tile([P, DK], F16)
    nc.sync.dma_start(sel[:], sel_ap)
    ident = consts.tile([P, P], F16)
    nc.sync.dma_start(ident[:], ident_ap)
    ones = consts.tile([P, 1], F16)
    nc.vector.memset(ones[:], 1.0)
    wf = consts.tile([P, KT, ND], F16)
    nc.scalar.copy(wf.rearrange("p t d -> p (t d)"), w32.rearrange("p t d -> p (t d)"))

    # ---- per-group routing state (memsets first: Pool FIFO runs these
    # before the big SWDGE loads so group 0 can start immediately) ----
    ct_tiles = [
        [sm.tile([P, JT, GS, 32], F16, name="ct") for _ in range(2)] for _ in range(NG)
    ]
    for g in range(NG):
        for tt in ct_tiles[g]:
            nc.gpsimd.memset(tt[:], 0.0)
        nc.gpsimd.memset(ct_tiles[g][0][:, :, :, 0:NI], 1.0 / NI)

    # ---- per-group x in both layouts, host-packed so every group slice is
    # 128 descriptors x 16KB contiguous (j is relabeled j = 8p + r on the
    # host; the routing is j-permutation invariant and xn/xt/ct/bT all use
    # the same labeling) ----
    xn_g = [None] * NG  # [P, GS, 8(r), NK]       xn[p,s,r,k'] = x[s, 8p+r, k']
    xt_g = [None] * NG  # [P, GS, JT, KT, P]      xt[q,s,r,kt,p] = x[s, 8p+r, 128kt+q]
    for g in range(NG):
        xn_g[g] = xp.tile([P, GS, JT, NK], F16, name="xn")
        xt_g[g] = xp.tile([P, GS, JT, KT, P], F16, name="xt")
        nc.gpsimd.dma_start(xn_g[g][:], x_ap[:, g * GS : (g + 1) * GS])
    # wT[p(ik%128), ikt, kt, q(k'%128)] = W[128kt+q, 128ikt+p]
    wT = consts.tile([P, IKT, KT, P], F16)
    for kt in range(KT):
        nc.sync.dma_start_transpose(wT[:, :, kt, :], wf[:, kt, :])

    def emit_xt(g):
        """Derive xt from xn via one SBUF->SBUF xbar transpose per sample
        (zero HBM, zero PE). out[p', 2r+kt, q] = xn[q, 256r + 128kt + p']."""
        for a in range(GS):
            nc.sync.dma_start_transpose(
                xt_g[g][:, a].rearrange("p r kt q -> p (r kt) q"),
                xn_g[g][:, a].rearrange("p r k -> p (r k)"),
            )

    emit_xt(0)

    rinv_g = [None] * NG
    maskedT_g = [None] * NG

    def emit_A(g, t):
        """cx = C x  (strip-packed, contract j), evacuate, DMA-transpose."""
        ct = ct_tiles[g][t % 2]
        pcx = psum.tile([P, NK], F32, name="pcx", tag="pcx", bufs=1)
        for jt in range(JT):
            for a in range(GS):
                nc.tensor.matmul(
                    pcx[ts(a, 32), :],
                    lhsT=ct[:, jt, a, :],
                    rhs=xn_g[g][:, a, jt, :],
                    start=(jt == 0),
                    stop=(jt == JT - 1),
                    tile_position=(0, 32 * a),
                    skip_group_check=True,
                )
        cx = rt.tile([P, NK], F16, name="cx")
        nc.vector.tensor_copy(cx[:], pcx[:])
        pcxT = psum.tile([P, KT * P], F16, name="pcxT", tag="pcxT", bufs=1)
        for kt in range(KT):
            nc.tensor.transpose(pcxT[:, ts(kt, P)], cx[:, ts(kt, P)], ident[:])
        cxT = rt.tile([P, KT, P], F16, name="cxT")
        nc.scalar.copy(cxT.rearrange("p t q -> p (t q)"), pcxT[:])
        return cxT

    def emit_C(g, t, cxT):
        """sT = W-chunk^T cxT directly in [ik-part, (a,i)] orientation; mask;
        norms via ones-matmul; rinv."""
        ps_sT = psum.tile([P, IKT, P], F32, name="ps_sT", tag="ps", bufs=2)
        for ikt in range(IKT):
            for kt in range(KT):
                nc.tensor.matmul(
                    ps_sT[:, ikt, :],
                    lhsT=wf[:, kt, ts(ikt, P)],
                    rhs=cxT[:, kt, :],
                    start=(kt == 0),
                    stop=(kt == KT - 1),
                    skip_group_check=True,
                )
        maskedT = rt.tile([P, IKT, P], F16, name="maskedT")
        nc.vector.tensor_tensor(
            maskedT.rearrange("p t q -> p (t q)"),
            ps_sT.rearrange("p t q -> p (t q)"),
            maskT.rearrange("p t q -> p (t q)"),
            op=ALU.mult,
        )
        mtsq = rt.tile([P, IKT, P], F16, name="mtsq")
        nc.gpsimd.tensor_tensor(
            mtsq.rearrange("p t q -> p (t q)"),
            maskedT.rearrange("p t q -> p (t q)"),
            maskedT.rearrange("p t q -> p (t q)"),
            op=ALU.mult,
        )
        pn2 = psum.tile([P, 1], F32, name="pn2", tag="pn2", bufs=1)
        for ikt in range(IKT):
            nc.tensor.matmul(
                pn2[:],
                lhsT=mtsq[:, ikt, :],
                rhs=ones[:],
                start=(ikt == 0),
                stop=(ikt == IKT - 1),
            )
        # rinv = (n2+eps)^-0.5 on DVE: magic-constant guess + 2 Newton steps
        xe = rt.tile([P, 1], F32, name="xe")
        nc.vector.tensor_scalar(xe[:], pn2[:], EPS, None, op0=ALU.add)
        xh = rt.tile([P, 1], F32, name="xh")
        nc.vector.tensor_scalar(xh[:], xe[:], 0.5, None, op0=ALU.mult)
        yt = rt.tile([P, 1], F32, name="yt")
        nc.vector.tensor_scalar(
            yt.bitcast(I32)[:], xe.bitcast(I32)[:], 1, None,
            op0=ALU.logical_shift_right,
        )
        nc.vector.tensor_scalar(
            yt.bitcast(I32)[:], yt.bitcast(I32)[:], 0x5F3759E0, None,
            op0=ALU.subtract,
        )
        nc.vector.tensor_scalar(
            yt.bitcast(I32)[:], yt.bitcast(I32)[:], -1, None,
            op0=ALU.bitwise_xor,
        )
        y2 = rt.tile([P, 1], F32, name="y2")
        for _ in range(2):
            nc.vector.tensor_tensor(y2[:], yt[:], yt[:], op=ALU.mult)
            nc.vector.tensor_tensor(y2[:], y2[:], xh[:], op=ALU.mult)
            nc.vector.tensor_scalar(y2[:], y2[:], -1.0, 1.5, op0=ALU.mult, op1=ALU.add)
            nc.vector.tensor_tensor(yt[:], yt[:], y2[:], op=ALU.mult)
        rinv_g[g] = yt
        maskedT_g[g] = maskedT

    def emit_wv_b(g, t):
        """w_v = W^T-blocks . v (tiny), then b = w_v . xT (strip-packed)."""
        maskedT = maskedT_g[g]
        ps_wv = psum.tile([P, KT * P], F32, name="ps_wv", tag="paux", bufs=1)
        for kt in range(KT):
            for ikt in range(IKT):
                nc.tensor.matmul(
                    ps_wv[:, ts(kt, P)],
                    lhsT=wT[:, ikt, kt, :],
                    rhs=maskedT[:, ikt, :],
                    start=(ikt == 0),
                    stop=(ikt == IKT - 1),
                    skip_group_check=True,
                )
        wv = rt.tile([P, KT, P], F16, name="wv")
        nc.scalar.copy(wv.rearrange("p t q -> p (t q)"), ps_wv[:])
        bsc = rt.tile([P, 2, ND], F16, name="bsc")
        for jc in range(2):
            ps_b = psum.tile([P, ND], F32, name="ps_b", tag="pb", bufs=2)
            for kt in range(KT):
                for a in range(GS):
                    nc.tensor.matmul(
                        ps_b[ts(a, 32), :],
                        lhsT=wv[:, kt, ts(a, 32)],
                        rhs=xt_g[g][:, a, 4 * jc : 4 * (jc + 1), kt, :],
                        start=(kt == 0),
                        stop=(kt == KT - 1),
                        tile_position=(0, 32 * a),
                        skip_group_check=True,
                    )
            nc.scalar.activation(bsc[:, jc, :], ps_b[:], AF.Exp, scale=rinv_g[g][:])
        return bsc

    def emit_soft(g, t, bsc):
        """bT via DMA transpose; softmax over i; write c_{t+1}."""
        bT = rt.tile([P, JT, P], F16, name="bT")
        nc.sync.dma_start_transpose(bT[:], bsc.rearrange("p c d -> p (c d)"))
        expT = bT.rearrange("p t (s c) -> p t s c", c=32)[:, :, :, 0:NI]
        zsum = rt.tile([P, JT, GS], F32, name="zsum")
        nc.vector.tensor_reduce(zsum[:], expT, axis=AX.X, op=ALU.add)
        rz = rt.tile([P, JT, GS], F32, name="rz")
        nc.vector.reciprocal(rz[:], zsum[:])
        ct_next = ct_tiles[g][(t + 1) % 2]
        nc.vector.tensor_tensor(
            ct_next[:, :, :, 0:NI],
            expT,
            rz.unsqueeze(3).broadcast_to([P, JT, GS, NI]),
            op=ALU.mult,
        )

    def emit_final(g):
        """diag-extract via sel matmul from maskedT, scale by rinv, DMA out."""
        maskedT = maskedT_g[g]
        ps_v = psum.tile([P, DK], F32, name="ps_v", tag="paux", bufs=1)
        for kt in range(IKT):
            nc.tensor.matmul(
                ps_v[:],
                lhsT=maskedT[:, kt, :],
                rhs=sel[:],
                start=(kt == 0),
                stop=(kt == IKT - 1),
            )
        vout = rt.tile([P, DK], F32, name="vout")
        nc.scalar.activation(vout[:], ps_v[:], AF.Copy, scale=rinv_g[g][:])
        for a in range(GS):
            nc.gpsimd.dma_start(
                out_ap[g * GS + a], vout[32 * a : 32 * a + NI, :]
            )

    # Diagonal wavefront: group g's iteration t belongs to wave w = g + t, so
    # g0's whole chain runs while g2/g3's x is still loading and no engine
    # FIFO is head-of-line blocked by not-yet-ready work. Within a wave, emit
    # stage-major so the 3-4 concurrent iterations' matmuls interleave on the
    # PE queue and hide each other's evac/DMA-transpose chain latency.
    for w in range(NG + ROUTINGS - 1):
        pairs = [(g, w - g) for g in range(NG) if 0 <= w - g < ROUTINGS]
        if w + 1 < NG:
            emit_xt(w + 1)
        cxTs = {g: emit_A(g, t) for g, t in pairs}
        for g, t in pairs:
            emit_C(g, t, cxTs[g])
        bscs = {g: emit_wv_b(g, t) for g, t in pairs if t < ROUTINGS - 1}
        for g, t in pairs:
            if t < ROUTINGS - 1:
                emit_soft(g, t, bscs[g])
            else:
                emit_final(g)


def _np_consts():
    # maskT[p, ikt, col]: 1 iff ik=128*ikt+p is in capsule block of col
    # (col = 32a + sigma, sigma = col % 32; block sigma = 4*ikt + p//32)
    maskT = np.zeros((P, IKT, P), dtype=np.float16)
    for p in range(P):
        for ikt in range(IKT):
            sig = 4 * ikt + p // 32
            for a in range(GS):
                maskT[p, ikt, 32 * a + sig] = 1.0
    sel = np.tile(np.eye(DK, dtype=np.float16), (IKT, 1))
    ident = np.eye(P, dtype=np.float16)
    return maskT, sel, ident


@functools.cache
def _build_nc():
    from contextlib import ExitStack

    nc = bacc.Bacc(
        "TRN2",
        target_bir_lowering=False,
        debug=False,
        num_devices=NCORES,
    )
    x_t = nc.dram_tensor("x", [P, BSH, JT, NK], F16, kind="ExternalInput")
    w_t = nc.dram_tensor("w", [NK, ND], F32, kind="ExternalInput")
    mask_t = nc.dram_tensor("mask", [P, IKT, P], F16, kind="ExternalInput")
    sel_t = nc.dram_tensor("sel", [P, DK], F16, kind="ExternalInput")
    ident_t = nc.dram_tensor("ident", [P, P], F16, kind="ExternalInput")
    out_t = nc.dram_tensor("out", [BSH, NI, DK], F32, kind="ExternalOutput")

    with tile.TileContext(nc) as tc:
        with ExitStack() as ctx:
            _build_body(
                nc,
                tc,
                x_t.ap(),
                w_t.ap(),
                mask_t.ap(),
                sel_t.ap(),
                ident_t.ap(),
                out_t.ap(),
                ctx,
            )
    nc.compile()
    return nc


def _in_maps(x, W):
    x = np.asarray(x, dtype=np.float32)
    w2d = np.ascontiguousarray(np.asarray(W, dtype=np.float32).reshape(NK, ND))
    maskT, sel, ident = _np_consts()
    x16 = x.astype(np.float16)
    maps = []
    for c in range(NCORES):
        xs = x16[c * BSH : (c + 1) * BSH]  # [BSH, NJ, NK]
        # j is relabeled j = 8p + r (16KB-contiguous group loads)
        xn = np.ascontiguousarray(
            xs.reshape(BSH, P, JT, NK).transpose(1, 0, 2, 3)
        )  # [P, BSH, 8, NK]; xn[p,s,r,:] = x[s, 8p+r, :]
        maps.append(
            {"x": xn, "w": w2d, "mask": maskT, "sel": sel, "ident": ident}
        )
    return maps


def run(x, W, trace=False):
    nc = _build_nc()
    res = run_bass_kernel_spmd(nc, _in_maps(x, W), list(range(NCORES)), trace=trace)
    out = np.concatenate([r["out"] for r in res.results], axis=0)
    return out.astype(np.float32), res


def kernel(x, W):
    out, _ = run(x, W, trace=False)
    return out


# revision 53
# speedup vs baseline: 1.2817x; 1.2817x over previous
"""CapsuleLayer dynamic-routing kernel for Trainium2 (Bass/Tile), SPMD over 8 cores.

Math (per batch sample, from the reference):
    u_hat[j, (i,k)] = sum_k' x[j, k'] * W[k', (i,k)]        j=1024, k'=256, (i,k)=16x32=512
    b_0 = 0
    for t in 0..3:
        c = softmax_i(b)                                    [16, 1024]
        s[i, k] = sum_j c[i, j] * u_hat[j, (i,k)]
        v = s / sqrt(sum_k s^2 + eps)                       [16, 32]
        if t < 3: b[i, j] = sum_k v[i, k] * u_hat[j, (i,k)]
    return v

KEY STRUCTURE: u_hat is never materialized. Both routing contractions factor
through W:
    s = (C x) W          (contract j against x [256 wide], then k'=256 with W)
    b = (W v) . x        (tiny w_v = W^T-blocks . v pass, then k'=256 with xT)
This removes the u_hat matmuls, all PE transposes of u_hat, and the PSUM
evacuation traffic that dominated the direct implementation.

Sharding: data-parallel over batch (128 -> 16 per core), W replicated.

Per-core schedule: 4 groups x 4 samples (col-strip packed, tile_position),
4 routing rounds emitted stage-major round-robin across groups so every
engine queue (PE / DVE / ScalarE / sync-DMA) sees work in readiness order.
All transposes (cx->cxT, masked->maskedT, b->bT) are DMA xbar transposes
(SBUF->SBUF) on the sync queue - zero PE time. xT is pre-transposed on the
host so both x layouts stream from DRAM with plain DMAs.
"""

import functools

import numpy as np

import concourse.bass as bass
import concourse.mybir as mybir
import concourse.tile as tile
from concourse import bacc
from concourse.bass_utils import run_bass_kernel_spmd

F32 = mybir.dt.float32
I32 = mybir.dt.int32
F16 = mybir.dt.float16
AF = mybir.ActivationFunctionType
ALU = mybir.AluOpType
AX = mybir.AxisListType
ts = bass.ts

NCORES = 8
BFULL = 128
BSH = BFULL // NCORES  # 16 samples per core
NJ, NK, ND = 1024, 256, 512  # j, k', (i,k)
NI, DK = 16, 32
JT, KT, IKT = NJ // 128, NK // 128, ND // 128  # 8, 2, 4
GS = 4  # samples per routing group (packed in PSUM partitions at 32-stride)
NG = BSH // GS  # 4
ROUTINGS = 4
EPS = 1e-7
P = 128


def _build_body(nc, tc, x_ap, xt_ap, w_ap, mask_ap, sel_ap, ident_ap, out_ap, ctx):
    consts = ctx.enter_context(tc.tile_pool(name="consts", bufs=1))
    xp = ctx.enter_context(tc.tile_pool(name="xp", bufs=NG))
    sm = ctx.enter_context(tc.tile_pool(name="sm", bufs=2 * NG))
    rt = ctx.enter_context(tc.tile_pool(name="rt", bufs=4))
    psum = ctx.enter_context(tc.tile_pool(name="psum", bufs=2, space="PSUM"))

    # ---- constants (w32 first: the wf cast gates wT + C-stage) ----
    w32 = consts.tile([P, KT, ND], F32)
    nc.sync.dma_start(w32[:], w_ap.rearrange("(t p) d -> p t d", p=P))
    maskT = consts.tile([P, IKT, P], F16)
    nc.sync.dma_start(maskT[:], mask_ap)
    sel = consts.tile([P, DK], F16)
    nc.sync.dma_start(sel[:], sel_ap)
    ident = consts.tile([P, P], F16)
    nc.sync.dma_start(ident[:], ident_ap)
    ones = consts.tile([P, 1], F16)
    nc.vector.memset(ones[:], 1.0)
    wf = consts.tile([P, KT, ND], F16)
    nc.scalar.copy(wf.rearrange("p t d -> p (t d)"), w32.rearrange("p t d -> p (t d)"))

    # ---- per-group routing state (memsets first: Pool FIFO runs these
    # before the big SWDGE loads so group 0 can start immediately) ----
    ct_tiles = [
        [sm.tile([P, JT, GS, 32], F16, name="ct") for _ in range(2)] for _ in range(NG)
    ]
    for g in range(NG):
        for tt in ct_tiles[g]:
            nc.gpsimd.memset(tt[:], 0.0)
        nc.gpsimd.memset(ct_tiles[g][0][:, :, :, 0:NI], 1.0 / NI)

    # ---- per-group x in both layouts, host-packed so every group slice is
    # 128 descriptors x 16KB contiguous (j is relabeled j = 8p + r on the
    # host; the routing is j-permutation invariant and xn/xt/ct/bT all use
    # the same labeling) ----
    # wT[p(ik%128), ikt, kt, q(k'%128)] = W[128kt+q, 128ikt+p]
    # (on the scalar hwdge queue: ready ~6us, before the first wv stage)
    wT = consts.tile([P, IKT, KT, P], F16)
    for kt in range(KT):
        nc.scalar.dma_start_transpose(wT[:, :, kt, :], wf[:, kt, :])

    # Loads in need-order. HWDGE (sync) moves ~2MB per ~5us of queue time;
    # group 3's pair goes on the gpsimd SWDGE queue, which starts at once
    # and finishes well before wave 3 needs it.
    xn_g = [None] * NG  # [P, GS, 8(r), NK]       xn[p,s,r,k'] = x[s, 8p+r, k']
    xt_g = [None] * NG  # [P, GS, JT, KT, P]      xt[q,s,r,kt,p] = x[s, 8p+r, 128kt+q]
    for g in range(NG):
        xn_g[g] = xp.tile([P, GS, JT, NK], F16, name="xn")
        xt_g[g] = xp.tile([P, GS, JT, KT, P], F16, name="xt")
    nc.gpsimd.dma_start(xn_g[3][:], x_ap[:, 3 * GS : 4 * GS])
    nc.gpsimd.dma_start(xt_g[3][:], xt_ap[:, 3 * GS : 4 * GS])
    for g in range(3):
        nc.sync.dma_start(xn_g[g][:], x_ap[:, g * GS : (g + 1) * GS])
        nc.sync.dma_start(xt_g[g][:], xt_ap[:, g * GS : (g + 1) * GS])

    rinv_g = [None] * NG
    maskedT_g = [None] * NG

    def emit_A(g, t):
        """cx = C x  (strip-packed, contract j), evacuate, DMA-transpose."""
        ct = ct_tiles[g][t % 2]
        pcx = psum.tile([P, NK], F32, name="pcx", tag="pcx", bufs=1)
        for jt in range(JT):
            for a in range(GS):
                nc.tensor.matmul(
                    pcx[ts(a, 32), :],
                    lhsT=ct[:, jt, a, :],
                    rhs=xn_g[g][:, a, jt, :],
                    start=(jt == 0),
                    stop=(jt == JT - 1),
                    tile_position=(0, 32 * a),
                    skip_group_check=True,
                )
        cx = rt.tile([P, NK], F16, name="cx")
        nc.vector.tensor_copy(cx[:], pcx[:])
        pcxT = psum.tile([P, KT * P], F16, name="pcxT", tag="pcxT", bufs=1)
        for kt in range(KT):
            nc.tensor.transpose(pcxT[:, ts(kt, P)], cx[:, ts(kt, P)], ident[:])
        cxT = rt.tile([P, KT, P], F16, name="cxT")
        nc.scalar.copy(cxT.rearrange("p t q -> p (t q)"), pcxT[:])
        return cxT

    def emit_C(g, t, cxT):
        """sT = W-chunk^T cxT directly in [ik-part, (a,i)] orientation; mask;
        norms via ones-matmul; rinv."""
        ps_sT = psum.tile([P, IKT, P], F32, name="ps_sT", tag="ps", bufs=2)
        for ikt in range(IKT):
            for kt in range(KT):
                nc.tensor.matmul(
                    ps_sT[:, ikt, :],
                    lhsT=wf[:, kt, ts(ikt, P)],
                    rhs=cxT[:, kt, :],
                    start=(kt == 0),
                    stop=(kt == KT - 1),
                    skip_group_check=True,
                )
        maskedT = rt.tile([P, IKT, P], F16, name="maskedT")
        nc.vector.tensor_tensor(
            maskedT.rearrange("p t q -> p (t q)"),
            ps_sT.rearrange("p t q -> p (t q)"),
            maskT.rearrange("p t q -> p (t q)"),
            op=ALU.mult,
        )
        mtsq = rt.tile([P, IKT, P], F16, name="mtsq")
        nc.gpsimd.tensor_tensor(
            mtsq.rearrange("p t q -> p (t q)"),
            maskedT.rearrange("p t q -> p (t q)"),
            maskedT.rearrange("p t q -> p (t q)"),
            op=ALU.mult,
        )
        pn2 = psum.tile([P, 1], F32, name="pn2", tag="pcxT", bufs=1)
        for ikt in range(IKT):
            nc.tensor.matmul(
                pn2[:],
                lhsT=mtsq[:, ikt, :],
                rhs=ones[:],
                start=(ikt == 0),
                stop=(ikt == IKT - 1),
            )
        # rinv = (n2+eps)^-0.5 on DVE: magic-constant guess + 2 Newton steps
        xe = rt.tile([P, 1], F32, name="xe")
        nc.vector.tensor_scalar(xe[:], pn2[:], EPS, None, op0=ALU.add)
        xh = rt.tile([P, 1], F32, name="xh")
        nc.vector.tensor_scalar(xh[:], xe[:], 0.5, None, op0=ALU.mult)
        yt = rt.tile([P, 1], F32, name="yt")
        nc.vector.tensor_scalar(
            yt.bitcast(I32)[:], xe.bitcast(I32)[:], 1, None,
            op0=ALU.logical_shift_right,
        )
        nc.vector.tensor_scalar(
            yt.bitcast(I32)[:], yt.bitcast(I32)[:], 0x5F3759E0, None,
            op0=ALU.subtract,
        )
        nc.vector.tensor_scalar(
            yt.bitcast(I32)[:], yt.bitcast(I32)[:], -1, None,
            op0=ALU.bitwise_xor,
        )
        y2 = rt.tile([P, 1], F32, name="y2")
        for _ in range(2):
            nc.vector.tensor_tensor(y2[:], yt[:], yt[:], op=ALU.mult)
            nc.vector.tensor_tensor(y2[:], y2[:], xh[:], op=ALU.mult)
            nc.vector.tensor_scalar(y2[:], y2[:], -1.0, 1.5, op0=ALU.mult, op1=ALU.add)
            nc.vector.tensor_tensor(yt[:], yt[:], y2[:], op=ALU.mult)
        rinv_g[g] = yt
        maskedT_g[g] = maskedT

    def emit_wv_b(g, t):
        """w_v = W^T-blocks . v (tiny), then b = w_v . xT (strip-packed)."""
        maskedT = maskedT_g[g]
        ps_wv = psum.tile([P, KT * P], F32, name="ps_wv", tag="paux", bufs=1)
        for kt in range(KT):
            for ikt in range(IKT):
                nc.tensor.matmul(
                    ps_wv[:, ts(kt, P)],
                    lhsT=wT[:, ikt, kt, :],
                    rhs=maskedT[:, ikt, :],
                    start=(ikt == 0),
                    stop=(ikt == IKT - 1),
                    skip_group_check=True,
                )
        wv = rt.tile([P, KT, P], F16, name="wv")
        nc.scalar.copy(wv.rearrange("p t q -> p (t q)"), ps_wv[:])
        bsc = rt.tile([P, 2, ND], F16, name="bsc")
        for jc in range(2):
            ps_b = psum.tile([P, ND], F32, name="ps_b", tag="pb", bufs=2)
            for kt in range(KT):
                for a in range(GS):
                    nc.tensor.matmul(
                        ps_b[ts(a, 32), :],
                        lhsT=wv[:, kt, ts(a, 32)],
                        rhs=xt_g[g][:, a, 4 * jc : 4 * (jc + 1), kt, :],
                        start=(kt == 0),
                        stop=(kt == KT - 1),
                        tile_position=(0, 32 * a),
                        skip_group_check=True,
                    )
            nc.scalar.activation(bsc[:, jc, :], ps_b[:], AF.Exp, scale=rinv_g[g][:])
        return bsc

    def emit_soft(g, t, bsc):
        """bT via PE transposes; softmax over i (reading PSUM); write c_{t+1}."""
        bT = psum.tile([P, JT, P], F16, name="pbt", tag="pbt", bufs=1)
        for jt in range(JT):
            nc.tensor.transpose(
                bT[:, jt, :], bsc[:, jt // 4, ts(jt % 4, P)], ident[:]
            )
        expT = bT.rearrange("p t (s c) -> p t s c", c=32)[:, :, :, 0:NI]
        zsum = rt.tile([P, JT, GS], F32, name="zsum")
        nc.vector.tensor_reduce(zsum[:], expT, axis=AX.X, op=ALU.add)
        rz = rt.tile([P, JT, GS], F32, name="rz")
        nc.vector.reciprocal(rz[:], zsum[:])
        ct_next = ct_tiles[g][(t + 1) % 2]
        nc.vector.tensor_tensor(
            ct_next[:, :, :, 0:NI],
            expT,
            rz.unsqueeze(3).broadcast_to([P, JT, GS, NI]),
            op=ALU.mult,
        )

    def emit_final(g):
        """diag-extract via sel matmul from maskedT, scale by rinv, DMA out."""
        maskedT = maskedT_g[g]
        ps_v = psum.tile([P, DK], F32, name="ps_v", tag="paux", bufs=1)
        for kt in range(IKT):
            nc.tensor.matmul(
                ps_v[:],
                lhsT=maskedT[:, kt, :],
                rhs=sel[:],
                start=(kt == 0),
                stop=(kt == IKT - 1),
            )
        vout = rt.tile([P, DK], F32, name="vout")
        nc.scalar.activation(vout[:], ps_v[:], AF.Copy, scale=rinv_g[g][:])
        for a in range(GS):
            nc.gpsimd.dma_start(
                out_ap[g * GS + a], vout[32 * a : 32 * a + NI, :]
            )

    # Diagonal wavefront: group g's iteration t belongs to wave w = g + t, so
    # g0's whole chain runs while g2/g3's x is still loading and no engine
    # FIFO is head-of-line blocked by not-yet-ready work. Within a wave, emit
    # stage-major so the 3-4 concurrent iterations' matmuls interleave on the
    # PE queue and hide each other's evac/DMA-transpose chain latency.
    for w in range(NG + ROUTINGS - 1):
        pairs = [(g, w - g) for g in range(NG) if 0 <= w - g < ROUTINGS]
        cxTs = {g: emit_A(g, t) for g, t in pairs}
        for g, t in pairs:
            emit_C(g, t, cxTs[g])
        bscs = {g: emit_wv_b(g, t) for g, t in pairs if t < ROUTINGS - 1}
        for g, t in pairs:
            if t < ROUTINGS - 1:
                emit_soft(g, t, bscs[g])
            else:
                emit_final(g)


def _np_consts():
    # maskT[p, ikt, col]: 1 iff ik=128*ikt+p is in capsule block of col
    # (col = 32a + sigma, sigma = col % 32; block sigma = 4*ikt + p//32)
    maskT = np.zeros((P, IKT, P), dtype=np.float16)
    for p in range(P):
        for ikt in range(IKT):
            sig = 4 * ikt + p // 32
            for a in range(GS):
                maskT[p, ikt, 32 * a + sig] = 1.0
    sel = np.tile(np.eye(DK, dtype=np.float16), (IKT, 1))
    ident = np.eye(P, dtype=np.float16)
    return maskT, sel, ident


@functools.cache
def _build_nc():
    from contextlib import ExitStack

    nc = bacc.Bacc(
        "TRN2",
        target_bir_lowering=False,
        debug=False,
        num_devices=NCORES,
    )
    x_t = nc.dram_tensor("x", [P, BSH, JT, NK], F16, kind="ExternalInput")
    xt_t = nc.dram_tensor("xt", [P, BSH, JT, KT, P], F16, kind="ExternalInput")
    w_t = nc.dram_tensor("w", [NK, ND], F32, kind="ExternalInput")
    mask_t = nc.dram_tensor("mask", [P, IKT, P], F16, kind="ExternalInput")
    sel_t = nc.dram_tensor("sel", [P, DK], F16, kind="ExternalInput")
    ident_t = nc.dram_tensor("ident", [P, P], F16, kind="ExternalInput")
    out_t = nc.dram_tensor("out", [BSH, NI, DK], F32, kind="ExternalOutput")

    with tile.TileContext(nc) as tc:
        with ExitStack() as ctx:
            _build_body(
                nc,
                tc,
                x_t.ap(),
                xt_t.ap(),
                w_t.ap(),
                mask_t.ap(),
                sel_t.ap(),
                ident_t.ap(),
                out_t.ap(),
                ctx,
            )
    nc.compile()
    return nc


def _in_maps(x, W):
    x = np.asarray(x, dtype=np.float32)
    w2d = np.ascontiguousarray(np.asarray(W, dtype=np.float32).reshape(NK, ND))
    maskT, sel, ident = _np_consts()
    x16 = x.astype(np.float16)
    maps = []
    for c in range(NCORES):
        xs = x16[c * BSH : (c + 1) * BSH]  # [BSH, NJ, NK]
        # j is relabeled j = 8p + r (16KB-contiguous group loads)
        xn = np.ascontiguousarray(
            xs.reshape(BSH, P, JT, NK).transpose(1, 0, 2, 3)
        )  # [P, BSH, 8, NK]; xn[p,s,r,:] = x[s, 8p+r, :]
        xt = np.ascontiguousarray(
            xs.reshape(BSH, P, JT, KT, P).transpose(4, 0, 2, 3, 1)
        )  # [P(q), BSH, r, kt, p]; xt[q,s,r,kt,p] = x[s, 8p+r, 128kt+q]
        maps.append(
            {"x": xn, "xt": xt, "w": w2d, "mask": maskT, "sel": sel, "ident": ident}
        )
    return maps


def run(x, W, trace=False):
    nc = _build_nc()
    res = run_bass_kernel_spmd(nc, _in_maps(x, W), list(range(NCORES)), trace=trace)
    out = np.concatenate([r["out"] for r in res.results], axis=0)
    return out.astype(np.float32), res


def kernel(x, W):
    out, _ = run(x, W, trace=False)
    return out
